# revision 1
# baseline (speedup 1.0000x reference)
"""Masked-attention kernel for 8 TRN2 NeuronCores (batch-parallel sharding).

Per-core shard: 2 batches of [S=2048, D=128] Q/K/V + [S, S] bool mask.
Layout strategy (per core):
  - scores are computed TRANSPOSED (S^T[k, q]) so the PV matmul consumes the
    exp() output directly with V in its natural [k, d] layout.
  - the mask is folded into the scores inside the PE accumulation: an extra
    matmul per (k-tile, q-subtile) with the mask chunk (DMA-cast u8->fp8e4)
    as the stationary operand and a -240*I fp8 identity as the moving
    operand; exp() then flushes masked entries to ~0.
  - softmax denominator: DVE accumulates exp tiles across k-tiles, then per
    q-subtile one [acc-chunk]^T @ ones matmul gives the denominator as a
    PSUM column; reciprocal on DVE; applied as a per-partition scalar after
    the final transpose.
  - Q^T/K^T and O^T->O transposes run on the TensorEngine (is_transpose
    matmuls vs a fp16 identity) through a small PSUM staging pool — the DMA
    xbar path was slower here because Tile serializes dma-transposes against
    in-flight DMAs.
  - Q/K loads ride the two HWDGE rings (SP + ACT) in halves, masks ride
    SWDGE (with the u8->fp8 cast) in 512-column chunks, so the first
    matmul can start ~10us in.
"""

import numpy as np
import ml_dtypes

B, S, D = 16, 2048, 128
NCORES = 8
BP = B // NCORES  # batches per core
P = 128
QC = 1024  # q-chunk (columns of the transposed score tile)
NQC = S // QC
NKT = S // P  # k tiles
NQS = QC // P  # q subtiles per chunk
MM_N = 512  # matmul moving free dim
SCALE = 1.0 / float(np.sqrt(128.0))
MASK_NEG = -240.0

_CACHE = {}


def build_nc(loop=True):
    import concourse.mybir as mybir
    import concourse.tile as tile
    from concourse import bacc

    fp16 = mybir.dt.float16
    fp32 = mybir.dt.float32

    nc = bacc.Bacc("TRN2", target_bir_lowering=False, debug=False,
                   num_devices=NCORES)

    Qd = nc.dram_tensor("Q", [BP, S, D], fp32, kind="ExternalInput")
    Kd = nc.dram_tensor("K", [BP, S, D], fp32, kind="ExternalInput")
    Vd = nc.dram_tensor("V", [BP, S, D], fp32, kind="ExternalInput")
    Md = nc.dram_tensor("mask", [BP, S, S], mybir.dt.uint8, kind="ExternalInput")
    if loop:
        # run-count knob for differential HW timing (graded path: loop=False)
        Id = nc.dram_tensor("iters", [1, 1], mybir.dt.int32,
                            kind="ExternalInput")
    Od = nc.dram_tensor("out", [BP, S, D], fp32, kind="ExternalOutput")

    negI_np = (MASK_NEG * np.eye(P, dtype=np.float32)).astype(
        ml_dtypes.float8_e4m3)
    negI_dram = nc.inline_tensor(negI_np, name="negI_const")
    ident_dram = nc.inline_tensor(np.eye(P, dtype=np.float16),
                                  name="ident_const")

    with tile.TileContext(nc) as tc:
        with tc.tile_pool(name="consts", bufs=1) as consts, \
             tc.tile_pool(name="stag", bufs=3) as stag, \
             tc.tile_pool(name="qkv", bufs=1) as qkv, \
             tc.tile_pool(name="maskp", bufs=6) as maskp, \
             tc.tile_pool(name="pp", bufs=3) as pp, \
             tc.tile_pool(name="accp", bufs=2) as accp, \
             tc.tile_pool(name="outp", bufs=2) as outp, \
             tc.tile_pool(name="spsum", bufs=2, space="PSUM") as spsum, \
             tc.tile_pool(name="opsum", bufs=1, space="PSUM") as opsum, \
             tc.tile_pool(name="tpsum", bufs=2, space="PSUM") as tpsum:

            negI = consts.tile([P, P], mybir.dt.float8e4)
            nc.sync.dma_start(out=negI[:, :], in_=negI_dram.ap())
            ident = consts.tile([P, P], fp16)
            nc.sync.dma_start(out=ident[:, :], in_=ident_dram.ap())
            ones_col = consts.tile([P, 1], fp16)
            nc.vector.memset(ones_col, 1.0)

            pools = (stag, qkv, maskp, pp, accp, outp, spsum, opsum, tpsum)
            if loop:
                it_sb = consts.tile([1, 1], mybir.dt.int32)
                nc.sync.dma_start(out=it_sb[:, :], in_=Id.ap())
                n_iters = nc.values_load(it_sb[:, :],
                                         skip_runtime_bounds_check=True)
                with tc.For_i(0, n_iters, 1,
                              hint_engines=(mybir.EngineType.PE,
                                            mybir.EngineType.Activation,
                                            mybir.EngineType.DVE,
                                            mybir.EngineType.SP,
                                            mybir.EngineType.Pool)):
                    _kernel_body(nc, mybir, Qd, Kd, Vd, Md, Od, negI,
                                 ident, ones_col, *pools)
            else:
                _kernel_body(nc, mybir, Qd, Kd, Vd, Md, Od, negI,
                             ident, ones_col, *pools)
    nc.compile()
    return nc


def _kernel_body(nc, mybir, Qd, Kd, Vd, Md, Od, negI, ident, ones_col,
                 stag, qkv, maskp, pp, accp, outp, spsum, opsum, tpsum):
    fp16 = mybir.dt.float16
    fp32 = mybir.dt.float32
    fp8 = mybir.dt.float8e4
    Exp = mybir.ActivationFunctionType.Exp

    MC = 512  # mask column-chunk (k) per DMA

    def load_mask_ck(b, qc, ck):
        # one tile per 512-column chunk: a single writer DMA, so the first
        # consuming matmul doesn't wait on later chunks (tile-granular deps)
        t = maskp.tile([P, NQS, MC], fp8, name="mfck")
        nc.gpsimd.dma_start(
            out=t[:, :, :],
            in_=Md.ap()[b, qc * QC:(qc + 1) * QC, ck * MC:(ck + 1) * MC]
                .rearrange("(s p) k -> p s k", p=P))
        return t

    # ---- prefetch the first mask columns before everything else (SWDGE) ----
    mf00 = [None] * (S // MC)
    mf00[0] = load_mask_ck(0, 0, 0)

    # ---- prep: load (HWDGE) + DVE-cast + PE-transpose Q/K, load V ----
    HT = NKT // 2  # tiles per half-load

    def load_f32_half(src_ap, b, h, ring):
        f = stag.tile([P, HT, D], fp32, name="ldf")
        ring(out=f[:, :, :],
             in_=src_ap[b, h * HT * P:(h + 1) * HT * P, :]
                 .rearrange("(t p) d -> p t d", p=P))
        return f

    def load_cast_half(src_ap, b, h, ring):
        # load a [S/2, D] f32 half and cast it to fp16 staging
        f = load_f32_half(src_ap, b, h, ring)
        g = stag.tile([P, HT, D], fp16, name="ldh")
        nc.vector.tensor_copy(out=g[:, :, :], in_=f[:, :, :])
        return g

    QT4 = HT // 2  # tiles per quarter

    def transpose_quarter(src_ap, dst, b, q4, ring):
        # finer first-quarter pipelining for the very first k-tiles;
        # dst is a single-writer per-quarter tile [P, 512]
        f = stag.tile([P, QT4, D], fp32, name="ldf4")
        ring(out=f[:, :, :],
             in_=src_ap[b, q4 * QT4 * P:(q4 + 1) * QT4 * P, :]
                 .rearrange("(t p) d -> p t d", p=P))
        g = stag.tile([P, QT4, D], fp16, name="ldh4")
        nc.vector.tensor_copy(out=g[:, :, :], in_=f[:, :, :])
        tps = tpsum.tile([P, QT4 * P], fp16, name="tps")
        for t in range(QT4):
            nc.tensor.transpose(tps[:, t * P:(t + 1) * P],
                                g[:, t, :], ident[:, :])
        nc.vector.tensor_copy(out=dst[:, :], in_=tps[:, :])

    def transpose_half(g, dstA, dstB):
        # PE-transpose a staged half into two per-quarter tiles
        tps = tpsum.tile([P, HT * P], fp16, name="tps")
        for t in range(HT):
            nc.tensor.transpose(tps[:, t * P:(t + 1) * P],
                                g[:, t, :], ident[:, :])
        nc.vector.tensor_copy(out=dstA[:, :], in_=tps[:, :QT4 * P])
        nc.vector.tensor_copy(out=dstB[:, :], in_=tps[:, QT4 * P:])

    def prep_batch(b):
        # per-half tiles: a consumer of h0 never waits on h1's writers.
        # h0: load+cast+transpose now; h1: f32 loads now, cast+transpose
        # deferred to mid-k-loop (finish()) so neither the in-order PE nor
        # the DVE FIFO head-of-line blocks the first k-loop.
        ktt = [qkv.tile([P, QT4 * P], fp16, name=f"ktt{b}{q4}")
               for q4 in range(4)]
        qt = [qkv.tile([P, QT4 * P], fp16, name=f"qt{b}{q4}")
              for q4 in range(4)]
        vsb = [qkv.tile([P, HT, D], fp16, name=f"vsb{b}{h}")
               for h in range(2)]

        def load_v_half(h):
            vf = stag.tile([P, HT, D], fp32, name="vf")
            nc.sync.dma_start(
                out=vf[:, :, :],
                in_=Vd.ap()[b, h * HT * P:(h + 1) * HT * P, :]
                    .rearrange("(t p) d -> p t d", p=P))
            nc.vector.tensor_copy(out=vsb[h][:, :, :], in_=vf[:, :, :])

        for q4 in range(2):
            transpose_quarter(Kd.ap(), ktt[q4], b, q4, nc.sync.dma_start)
            transpose_quarter(Qd.ap(), qt[q4], b, q4, nc.scalar.dma_start)
        load_v_half(0)
        if b == 0:
            mf00[1] = load_mask_ck(0, 0, 1)
            mf00[2] = load_mask_ck(0, 0, 2)
        fk1 = load_f32_half(Kd.ap(), b, 1, nc.sync.dma_start)
        if b == 0:
            mf00[3] = load_mask_ck(0, 0, 3)
        load_v_half(1)
        fq1 = load_f32_half(Qd.ap(), b, 1, nc.scalar.dma_start)

        state = {}

        def finish_cast():
            gk1 = stag.tile([P, HT, D], fp16, name="ldh")
            nc.vector.tensor_copy(out=gk1[:, :, :], in_=fk1[:, :, :])
            gq1 = stag.tile([P, HT, D], fp16, name="ldh")
            nc.vector.tensor_copy(out=gq1[:, :, :], in_=fq1[:, :, :])
            state["g"] = (gk1, gq1)

        def finish_transpose():
            gk1, gq1 = state["g"]
            transpose_half(gk1, ktt[2], ktt[3])
            transpose_half(gq1, qt[2], qt[3])
        return qt, ktt, vsb, (finish_cast, finish_transpose)

    prepped = {0: prep_batch(0)}
    finished = set()

    # ---- main flash loop over (batch, q-chunk, k-tile) ----
    for b in range(BP):
        for qc in range(NQC):
            if (b, qc) == (0, 1) and BP > 1:
                prepped[1] = prep_batch(1)
            qt, ktt, vsb, finish_fns = prepped[b]
            if b == 0 and qc == 0:
                mf = mf00
            else:
                mf = [load_mask_ck(b, qc, ck) for ck in range(S // MC)]
            acc = accp.tile([P, QC], fp16, name="acc")
            ops = opsum.tile([P, QC], fp32, name="opsum")
            for kt in range(NKT):
                if kt == HT - 3 and b not in finished:
                    finish_fns[0]()
                if kt == HT - 1 and b not in finished:
                    finish_fns[1]()
                    finished.add(b)
                sc = spsum.tile([P, QC], fp32, name="scores")
                mfck = mf[kt * P // MC]
                kcol = (kt * P) % MC
                for sq in range(NQS):
                    # start=True only on the first matmul touching each PSUM
                    # bank (start clears the whole bank's has_written bits)
                    nc.tensor.matmul(
                        sc[:, sq * P:(sq + 1) * P],
                        lhsT=mfck[:, sq, kcol:kcol + P],
                        rhs=negI[:, :],
                        start=(sq % (MM_N // P) == 0), stop=False,
                        skip_group_check=True)
                kh, kloc = kt // QT4, (kt % QT4) * P
                for n in range(0, QC, MM_N):
                    nc.tensor.matmul(
                        sc[:, n:n + MM_N],
                        lhsT=ktt[kh][:, kloc:kloc + P],
                        rhs=qt[qc * 2 + n // MM_N][:, :],
                        start=False, stop=True, skip_group_check=True)
                pt = pp.tile([P, QC], fp16, name="pt")
                nc.scalar.activation(out=pt[:, :], in_=sc[:, :],
                                     func=Exp, scale=SCALE)
                if kt == 0:
                    nc.vector.tensor_copy(out=acc[:, :], in_=pt[:, :])
                else:
                    nc.vector.tensor_add(out=acc[:, :], in0=acc[:, :],
                                         in1=pt[:, :])
                # PV lags one k-tile so the PE never waits on exp(kt)
                if kt > 0:
                    j = kt - 1
                    for n in range(0, QC, MM_N):
                        nc.tensor.matmul(
                            ops[:, n:n + MM_N],
                            lhsT=vsb[j // HT][:, j % HT, :],
                            rhs=prev_pt[:, n:n + MM_N],
                            start=(kt == 1), stop=False,
                            skip_group_check=True)
                prev_pt = pt
            j = NKT - 1
            for n in range(0, QC, MM_N):
                nc.tensor.matmul(
                    ops[:, n:n + MM_N],
                    lhsT=vsb[j // HT][:, j % HT, :],
                    rhs=prev_pt[:, n:n + MM_N],
                    start=False, stop=True,
                    skip_group_check=True)

            # denominator as a PSUM column per q-subtile:
            # den[q_local, sq] = sum_k acc[k, sq*128 + q_local]
            den = tpsum.tile([P, NQS], fp32, name="tps")
            for sq in range(NQS):
                nc.tensor.matmul(den[:, sq:sq + 1],
                                 lhsT=acc[:, sq * P:(sq + 1) * P],
                                 rhs=ones_col[:, :],
                                 start=True, stop=True,
                                 skip_group_check=True)
            rcol = outp.tile([P, NQS], fp32, name="rcol")
            nc.vector.reciprocal(out=rcol[:, :], in_=den[:, :])

            # epilogue in two 512-col halves so copy/transpose/scale/store
            # pipeline (shorter serial tail on the final chunk)
            HQ = NQS // 2
            for hh in range(2):
                ot = outp.tile([P, HQ * P], fp16, name="ot")
                nc.scalar.copy(out=ot[:, :],
                               in_=ops[:, hh * HQ * P:(hh + 1) * HQ * P])
                osb = tpsum.tile([P, HQ * P], fp16, name="tps")
                for t in range(HQ):
                    nc.tensor.transpose(osb[:, t * P:(t + 1) * P],
                                        ot[:, t * P:(t + 1) * P],
                                        ident[:, :])
                osf = outp.tile([P, HQ, D], fp32, name="osf")
                for t in range(HQ):
                    nc.vector.tensor_scalar_mul(
                        out=osf[:, t, :],
                        in0=osb[:, t * P:(t + 1) * P],
                        scalar1=rcol[:, hh * HQ + t:hh * HQ + t + 1])
                ring = nc.scalar.dma_start if hh == 0 else nc.sync.dma_start
                ring(out=Od.ap()[b,
                                 qc * QC + hh * HQ * P:
                                 qc * QC + (hh + 1) * HQ * P, :]
                     .rearrange("(t p) d -> p t d", p=P),
                     in_=osf[:, :, :])


def _get_nc(loop=False):
    key = f"nc_loop{loop}"
    if key not in _CACHE:
        _CACHE[key] = build_nc(loop=loop)
    return _CACHE[key]


def kernel(Q, K, V, mask, dk=128):
    from concourse.bass_utils import run_bass_kernel_spmd

    assert int(dk) == 128
    Q = np.ascontiguousarray(np.asarray(Q, dtype=np.float32))
    K = np.ascontiguousarray(np.asarray(K, dtype=np.float32))
    V = np.ascontiguousarray(np.asarray(V, dtype=np.float32))
    mask_u8 = np.ascontiguousarray(np.asarray(mask)).astype(np.uint8)

    nc = _get_nc(loop=False)
    in_maps = []
    for c in range(NCORES):
        sl = slice(c * BP, (c + 1) * BP)
        in_maps.append({
            "Q": np.ascontiguousarray(Q[sl]),
            "K": np.ascontiguousarray(K[sl]),
            "V": np.ascontiguousarray(V[sl]),
            "mask": np.ascontiguousarray(mask_u8[sl]),
        })
    res = run_bass_kernel_spmd(nc, in_maps, core_ids=list(range(NCORES)))
    return np.concatenate([r["out"] for r in res.results], axis=0)



# revision 22
# speedup vs baseline: 2.9873x; 2.9873x over previous
"""Masked-attention kernel for 8 TRN2 NeuronCores (batch-parallel sharding).

v3 design (host-assisted layouts, deferred epilogue, PE/Act/DVE balanced):
  - Host pre-transposes Q/K to [D, S] fp16, packs V as [P, NKT, D] fp16
    (partition-major so DMA runs are 2KB), and packs the mask TRANSPOSED
    as fp8e4m3 (0.0 / 1.0) in [S_k, S_q] layout. No device-side casts or
    prep transposes remain.
  - Scores per k-tile are computed transposed (sc[k, q]): stationary K^T
    tile vs the moving Q^T chunk. The mask bias (-240 * m) folds into the
    same PSUM accumulation with fp8 DoubleRow matmuls at 0.5 cycles/row:
    constants [negI || 0] and [0 || negI] select plane 0/1 of a
    [128, 2, 1024] mask tile, so one mask DMA feeds two k-tiles.
  - exp() on Act is the floor (~1.04us per [128,1024] tile).
  - DVE accumulates exp tiles (fp16 2x); 8 tiny PE matmuls vs a ones
    column give per-q denominators; DVE reciprocal; applied after the
    epilogue transpose as a per-partition scalar.
  - PV: V tile stationary, exp output moving, o^T accumulated in PSUM
    with a 2-tile lag so the PE never waits on exp.
  - Epilogue of chunk c is deferred into chunk c+1's first k-iterations
    (den@kt1, PSUM copy on GPSIMD@kt2, transposes@kt3, scales + output
    DMA@kt3) so the PE pipeline never drains at chunk boundaries.
  - Output is written fp16 in a partition-major packed layout
    [qc, p, t, d]; the host unpacks and casts to fp32.
"""

import numpy as np
import ml_dtypes

B, S, D = 16, 2048, 128
NCORES = 8
BP = B // NCORES  # batches per core
P = 128
QC = 1024  # q-chunk (columns of the transposed score tile)
NQC = S // QC
NKT = S // P  # k tiles
NQS = QC // P  # q subtiles per chunk
HKT = NKT // 2  # k tiles per half-load
SCALE = 1.0 / float(np.sqrt(128.0))
MASK_NEG = -240.0
PVLAG = 3
# mask application split: PE DoubleRow pairs take PE_TILES (plane0/plane1
# per pair), DVE post-exp multiply takes DVE_TILES
PE_TILES = [0, 2, 4, 6, 8, 10, 12, 14, 1, 3]
DVE_TILES = [5, 7, 9, 11, 13, 15]
PAIR_OF = {kt: (i // 2, i % 2) for i, kt in enumerate(PE_TILES)}
DIDX_OF = {kt: i for i, kt in enumerate(DVE_TILES)}
NPAIR = len(PE_TILES) // 2
# last chunk runs DVE-masked tiles first so its tail is mult-free
ORDER_LAST = [5, 7, 1, 3, 9, 11, 13, 15, 0, 2, 4, 6, 8, 10, 12, 14]


def _res_key(kt):
    if kt in PAIR_OF:
        return ("mt", PAIR_OF[kt][0])
    return ("nm", DIDX_OF[kt])


def _res_plan(order):
    seen, plan = set(), [[] for _ in range(NKT)]
    for i, kt in enumerate(order):
        k = _res_key(kt)
        if k not in seen:
            seen.add(k)
            plan[i].append(k)
    return plan

_CACHE = {}


def build_nc(loop=True):
    import concourse.mybir as mybir
    import concourse.tile as tile
    from concourse import bacc

    fp16 = mybir.dt.float16
    fp8 = mybir.dt.float8e4

    nc = bacc.Bacc("TRN2", target_bir_lowering=False, debug=False,
                   num_devices=NCORES)

    QTd = nc.dram_tensor("QT", [BP, D, S], fp16, kind="ExternalInput")
    KTd = nc.dram_tensor("KT", [BP, D, S], fp16, kind="ExternalInput")
    Vd = nc.dram_tensor("V", [BP, P, NKT, D], fp16, kind="ExternalInput")
    Md = nc.dram_tensor("MT", [BP, len(PE_TILES) * P, S], fp8,
                        kind="ExternalInput")
    NMd = nc.dram_tensor("NMT", [BP, len(DVE_TILES) * P, S], fp16,
                         kind="ExternalInput")
    if loop:
        Id = nc.dram_tensor("iters", [1, 1], mybir.dt.int32,
                            kind="ExternalInput")
    Od = nc.dram_tensor("out", [BP, NQC, P, NQS, D], fp16,
                        kind="ExternalOutput")

    # DoubleRow mask-bias weights: plane-selecting [negI || 0] / [0 || negI]
    w0_np = np.zeros((P, 2, P), dtype=np.float32)
    w0_np[:, 0, :] = MASK_NEG * np.eye(P, dtype=np.float32)
    w1_np = np.zeros((P, 2, P), dtype=np.float32)
    w1_np[:, 1, :] = MASK_NEG * np.eye(P, dtype=np.float32)
    w0_dram = nc.inline_tensor(w0_np.astype(ml_dtypes.float8_e4m3),
                               name="w0_const")
    w1_dram = nc.inline_tensor(w1_np.astype(ml_dtypes.float8_e4m3),
                               name="w1_const")
    ident_dram = nc.inline_tensor(np.eye(P, dtype=np.float16),
                                  name="ident_const")

    with tile.TileContext(nc) as tc:
        with tc.tile_pool(name="consts", bufs=1) as consts, \
             tc.tile_pool(name="qkv", bufs=1) as qkv, \
             tc.tile_pool(name="qtp", bufs=2) as qtp, \
             tc.tile_pool(name="maskp", bufs=3) as maskp, \
             tc.tile_pool(name="nmp", bufs=4) as nmp, \
             tc.tile_pool(name="pp", bufs=8) as pp, \
             tc.tile_pool(name="accp", bufs=2) as accp, \
             tc.tile_pool(name="outp", bufs=2) as outp, \
             tc.tile_pool(name="spsum", bufs=2, space="PSUM") as spsum, \
             tc.tile_pool(name="opsum", bufs=1, space="PSUM") as opsum, \
             tc.tile_pool(name="tpsum", bufs=1, space="PSUM") as tpsum:

            # w0 first on the HWDGE ring: it gates the first mask matmul
            w0 = consts.tile([P, 2, P], fp8)
            nc.sync.dma_start(out=w0[:, :, :], in_=w0_dram.ap())
            w1 = consts.tile([P, 2, P], fp8)
            ident = consts.tile([P, P], fp16)
            nc.gpsimd.dma_start(out=ident[:, :], in_=ident_dram.ap())
            ones_col = consts.tile([P, 1], fp16)
            nc.vector.memset(ones_col, 1.0)

            pools = (qkv, qtp, maskp, nmp, pp, accp, outp, spsum, opsum, tpsum)
            if loop:
                it_sb = consts.tile([1, 1], mybir.dt.int32)
                nc.sync.dma_start(out=it_sb[:, :], in_=Id.ap())
                n_iters = nc.values_load(it_sb[:, :],
                                         skip_runtime_bounds_check=True)
                with tc.For_i(0, n_iters, 1,
                              hint_engines=(mybir.EngineType.PE,
                                            mybir.EngineType.Activation,
                                            mybir.EngineType.DVE,
                                            mybir.EngineType.SP,
                                            mybir.EngineType.Pool)):
                    _kernel_body(nc, mybir, QTd, KTd, Vd, Md, NMd, Od,
                                 w0, w1, w1_dram.ap(), ident, ones_col,
                                 *pools)
            else:
                _kernel_body(nc, mybir, QTd, KTd, Vd, Md, NMd, Od,
                             w0, w1, w1_dram.ap(), ident, ones_col,
                             *pools)
    nc.compile()
    return nc


def _kernel_body(nc, mybir, QTd, KTd, Vd, Md, NMd, Od, w0, w1,
                 w1_dram_ap, ident, ones_col,
                 qkv, qtp, maskp, nmp, pp, accp, outp, spsum, opsum,
                 tpsum):
    fp16 = mybir.dt.float16
    fp32 = mybir.dt.float32
    fp8 = mybir.dt.float8e4
    Exp = mybir.ActivationFunctionType.Exp

    def load_mask_pair(b, qc, j):
        # [128, 2, 1024] fp8: plane 0 = k-tile 4j, plane 1 = k-tile 4j+2
        # (Md holds only the even k-tiles' mask rows, transposed)
        t = maskp.tile([P, 2, QC], fp8, name="mtile")
        nc.sync.dma_start(
            out=t[:, :, :],
            in_=Md.ap()[b, j * 2 * P:(j + 1) * 2 * P,
                        qc * QC:(qc + 1) * QC]
                .rearrange("(two p) q -> p two q", two=2))
        return t

    def load_nm(b, qc, o):
        # [128, 1024] fp16 not-mask for odd k-tile 2o+1
        t = nmp.tile([P, QC], fp16, name="nmtile")
        nc.sync.dma_start(
            out=t[:, :],
            in_=NMd.ap()[b, o * P:(o + 1) * P, qc * QC:(qc + 1) * QC])
        return t

    def load_qt(b, qc):
        t = qtp.tile([P, QC], fp16, name="qt")
        nc.sync.dma_start(out=t[:, :],
                            in_=QTd.ap()[b, :, qc * QC:(qc + 1) * QC])
        return t

    def load_k_half(b, h):
        t = qkv.tile([P, HKT * P], fp16, name=f"kt{b}{h}")
        nc.sync.dma_start(
            out=t[:, :], in_=KTd.ap()[b, :, h * HKT * P:(h + 1) * HKT * P])
        return t

    def load_v_half(b, h):
        t = qkv.tile([P, HKT, D], fp16, name=f"v{b}{h}")
        nc.sync.dma_start(
            out=t[:, :, :], in_=Vd.ap()[b, :, h * HKT:(h + 1) * HKT, :])
        return t

    mt_next = {}
    nm_next = {}

    def get_mt(b, qc, j):
        t = mt_next.get((b, qc, j))
        if t is None:
            t = mt_next[(b, qc, j)] = load_mask_pair(b, qc, j)
        return t

    def get_nm(b, qc, o):
        t = nm_next.get((b, qc, o))
        if t is None:
            t = nm_next[(b, qc, o)] = load_nm(b, qc, o)
        return t

    def get_res(b, qc, key):
        if key[0] == "mt":
            get_mt(b, qc, key[1])
        else:
            get_nm(b, qc, key[1])

    # startup: strict need-order so the single HWDGE unit serves the
    # critical path first (mask pairs 0/4, first K tile, Q^T chunk)
    get_mt(0, 0, 0)
    kt00 = qkv.tile([P, P], fp16, name="kt00")
    nc.scalar.dma_start(out=kt00[:, :], in_=KTd.ap()[0, :, 0:P])
    qt0 = qtp.tile([P, QC], fp16, name="qt")
    nc.scalar.dma_start(out=qt0[:, :], in_=QTd.ap()[0, :, 0:QC])
    qt_next = {(0, 0): qt0}
    get_mt(0, 0, 4)
    kt0x = {0: kt00}
    for x in (1, 2):
        t = qkv.tile([P, P], fp16, name=f"kt0{x}")
        nc.scalar.dma_start(out=t[:, :], in_=KTd.ap()[0, :, x * P:(x + 1) * P])
        kt0x[x] = t
    kv = {0: [load_k_half(0, 0)]}
    nc.sync.dma_start(out=w1[:, :, :], in_=w1_dram_ap)
    kv[0].append(load_v_half(0, 0))
    kv[0].append(load_k_half(0, 1))
    kv[0].append(load_v_half(0, 1))

    pend = {}
    pend_pv = []

    def emit_pv(ops, pts, jj, vh0, vh1, first, final):
        vsel = vh0 if jj < HKT else vh1
        for n in range(0, QC, 512):
            nc.tensor.matmul(
                ops[:, n:n + 512],
                lhsT=vsel[:, jj % HKT, :],
                rhs=pts[jj][:, n:n + 512],
                start=first, stop=final,
                skip_group_check=True)
        del pts[jj]

    def epi_den(c):
        acc, _, b, qc = pend[c]
        den = tpsum.tile([P, NQS], fp32, name="den")
        for sq in range(NQS):
            nc.tensor.matmul(den[:, sq:sq + 1],
                             lhsT=acc[:, sq * P:(sq + 1) * P],
                             rhs=ones_col[:, :],
                             start=True, stop=True,
                             skip_group_check=True)
        rcol = outp.tile([P, NQS], fp32, name="rcol")
        nc.vector.reciprocal(out=rcol[:, :], in_=den[:, :])
        pend[c] += (rcol,)

    def epi_copy(c, last=False):
        _, ops, b, qc, _ = pend[c]
        # PSUM drain on DVE (GPSIMD cannot access PSUM on real HW); in
        # the final flush Act is already idle, so it takes half
        ot = outp.tile([P, QC], fp16, name="ot")
        if last:
            H = QC // 2
            nc.vector.tensor_copy(out=ot[:, :H], in_=ops[:, :H])
            nc.scalar.copy(out=ot[:, H:], in_=ops[:, H:])
        else:
            nc.vector.tensor_copy(out=ot[:, :], in_=ops[:, :])
        pend[c] += (ot,)

    def epi_out(c, last=False):
        _, _, b, qc, rcol, ot = pend.pop(c)
        osb = tpsum.tile([P, QC], fp16, name="osb")
        osf = outp.tile([P, NQS, D], fp16, name="osf")
        HQ = NQS // 2
        for hh in range(2):
            for t in range(hh * HQ, (hh + 1) * HQ):
                nc.tensor.transpose(osb[:, t * P:(t + 1) * P],
                                    ot[:, t * P:(t + 1) * P],
                                    ident[:, :])
            for t in range(hh * HQ, (hh + 1) * HQ):
                if last and t % 2 == 1:
                    nc.scalar.activation(
                        out=osf[:, t, :],
                        in_=osb[:, t * P:(t + 1) * P],
                        func=mybir.ActivationFunctionType.Copy,
                        scale=rcol[:, t:t + 1])
                else:
                    nc.vector.tensor_scalar_mul(
                        out=osf[:, t, :],
                        in0=osb[:, t * P:(t + 1) * P],
                        scalar1=rcol[:, t:t + 1])
            ring = (nc.sync.dma_start if (hh == 0 or last)
                    else nc.gpsimd.dma_start)
            ring(out=Od.ap()[b, qc, :, hh * HQ:(hh + 1) * HQ, :],
                 in_=osf[:, hh * HQ:(hh + 1) * HQ, :])

    for b in range(BP):
        for qc in range(NQC):
            c = b * NQC + qc
            kh0, vh0, kh1, vh1 = kv[b]
            qt = qt_next.pop((b, qc))
            if qc + 1 < NQC:
                nb, nqc = b, qc + 1
            elif b + 1 < BP:
                nb, nqc = b + 1, 0
            else:
                nb = None
            acc = accp.tile([P, QC], fp16, name="acc")
            ops = opsum.tile([P, QC], fp32, name="opsum")
            pts = {}
            last = nb is None
            order = ORDER_LAST if last else list(range(NKT))
            res_plan = _res_plan(order)
            if nb is not None:
                next_last = (nb == BP - 1 and nqc == NQC - 1)
                next_plan = _res_plan(
                    ORDER_LAST if next_last else range(NKT))
            for i, kt in enumerate(order):
                sc = spsum.tile([P, QC], fp32, name="scores")
                if kt in PAIR_OF:
                    j, plane = PAIR_OF[kt]
                    mt = get_mt(b, qc, j)
                    wsel = w0 if plane == 0 else w1
                    for n in range(0, QC, 512):
                        nc.tensor.matmul(
                            sc[:, n:n + 512],
                            lhsT=wsel[:, :, :],
                            rhs=mt[:, :, n:n + 512],
                            start=True, stop=False,
                            perf_mode=mybir.MatmulPerfMode.DoubleRow,
                            skip_group_check=True)
                    qk_start = False
                else:
                    nm = get_nm(b, qc, DIDX_OF[kt])
                    qk_start = True
                if c == 0 and kt in kt0x and i < 3:
                    ksel, kloc = kt0x[kt], 0
                else:
                    ksel = kh0 if kt < HKT else kh1
                    kloc = (kt % HKT) * P
                for n in range(0, QC, 512):
                    nc.tensor.matmul(
                        sc[:, n:n + 512],
                        lhsT=ksel[:, kloc:kloc + P],
                        rhs=qt[:, n:n + 512],
                        start=qk_start, stop=True, skip_group_check=True)

                # previous chunk's PV tail + deferred epilogue, placed
                # AFTER this kt's QK so the Act pipeline never bubbles;
                # epi_copy precedes PV(c, 0) (single-buffered ops PSUM)
                if pend_pv:
                    if i == 0:
                        emit_pv(*pend_pv.pop(0))
                        emit_pv(*pend_pv.pop(0))
                    elif i == 1:
                        emit_pv(*pend_pv.pop(0))
                if c - 1 in pend:
                    if i == 1:
                        epi_den(c - 1)
                    elif i == 2:
                        epi_copy(c - 1)
                    elif i == 4:
                        epi_out(c - 1)

                pt = pp.tile([P, QC], fp16, name="pt")
                nc.scalar.activation(out=pt[:, :], in_=sc[:, :],
                                     func=Exp, scale=SCALE)
                if kt in PAIR_OF:
                    pts[kt] = pt
                else:
                    # apply the mask as an exact post-exp zeroing on DVE
                    pm = pts[kt] = pp.tile([P, QC], fp16, name="pm")
                    nc.vector.tensor_mul(out=pm[:, :], in0=pt[:, :],
                                         in1=nm[:, :])
                if i == 1:
                    nc.vector.tensor_add(out=acc[:, :],
                                         in0=pts[order[0]][:, :],
                                         in1=pts[order[1]][:, :])
                elif i > 1:
                    nc.vector.tensor_add(out=acc[:, :], in0=acc[:, :],
                                         in1=pts[kt][:, :])

                # prefetches (after compute emission so they never gate
                # it): stay ~3 k-tiles ahead in resource consumption order
                if i + 3 < NKT:
                    for key in res_plan[i + 3]:
                        get_res(b, qc, key)
                if i >= NKT - 3 and nb is not None:
                    for key in next_plan[i - (NKT - 3)]:
                        get_res(nb, nqc, key)
                if i == 6 and nb is not None:
                    qt_next[(nb, nqc)] = load_qt(nb, nqc)
                if nb is not None and nqc == 0:
                    if i == 8:
                        kv[nb] = [load_k_half(nb, 0), load_v_half(nb, 0)]
                    elif i == 10:
                        kv[nb] += [load_k_half(nb, 1), load_v_half(nb, 1)]

                # PV lags PVLAG k-tiles so the PE never waits on exp
                if i >= PVLAG:
                    emit_pv(ops, pts, order[i - PVLAG], vh0, vh1,
                            first=(i == PVLAG), final=False)
            tail = order[NKT - PVLAG:]
            for x, jj in enumerate(tail):
                pend_pv.append((ops, pts, jj, vh0, vh1, False,
                                x == len(tail) - 1))
            pend[c] = (acc, ops, b, qc)

    # final flush (no next chunk to hide it in)
    while pend_pv:
        emit_pv(*pend_pv.pop(0))
    c = BP * NQC - 1
    epi_den(c)
    epi_copy(c, last=True)
    epi_out(c, last=True)


def _get_nc(loop=False):
    key = f"nc_loop{loop}"
    if key not in _CACHE:
        _CACHE[key] = build_nc(loop=loop)
    return _CACHE[key]


def make_in_maps(Q, K, V, mask):
    """Host-side shard + layout prep: per-core input dicts."""
    Q = np.asarray(Q, dtype=np.float32)
    K = np.asarray(K, dtype=np.float32)
    V = np.asarray(V, dtype=np.float32)
    mask_b = np.asarray(mask).astype(bool)
    in_maps = []
    for c in range(NCORES):
        sl = slice(c * BP, (c + 1) * BP)
        qt = np.ascontiguousarray(
            Q[sl].transpose(0, 2, 1)).astype(np.float16)
        kt = np.ascontiguousarray(
            K[sl].transpose(0, 2, 1)).astype(np.float16)
        # V packed partition-major: [BP, P, NKT, D]
        v16 = np.ascontiguousarray(
            V[sl].reshape(BP, NKT, P, D).transpose(0, 2, 1, 3)
        ).astype(np.float16)
        mT = np.ascontiguousarray(mask_b[sl].transpose(0, 2, 1))
        mT4 = mT.reshape(BP, NKT, P, S)
        mt8 = np.ascontiguousarray(mT4[:, PE_TILES]).reshape(
            BP, len(PE_TILES) * P, S).astype(ml_dtypes.float8_e4m3)
        nmt = np.ascontiguousarray(~mT4[:, DVE_TILES]).reshape(
            BP, len(DVE_TILES) * P, S).astype(np.float16)
        in_maps.append({"QT": qt, "KT": kt, "V": v16, "MT": mt8,
                        "NMT": nmt})
    return in_maps


def unpack_out(raw):
    """[BP, NQC, P, NQS, D] fp16 -> [BP, S, D] fp32."""
    return np.ascontiguousarray(
        raw.transpose(0, 1, 3, 2, 4)).reshape(BP, S, D).astype(np.float32)


def kernel(Q, K, V, mask, dk=128):
    from concourse.bass_utils import run_bass_kernel_spmd

    assert int(dk) == 128
    nc = _get_nc(loop=False)
    in_maps = make_in_maps(Q, K, V, mask)
    res = run_bass_kernel_spmd(nc, in_maps, core_ids=list(range(NCORES)))
    return np.concatenate([unpack_out(r["out"]) for r in res.results],
                          axis=0)
